# revision 2
# baseline (speedup 1.0000x reference)
"""CBOW embedding-lookup kernel for Trainium2 (8 NeuronCores).

Math: out[b, o] = sum_i fc_w[o, i*V + contexts[b, i]] + fc_b[o]
i.e. a row-gather over a transposed view of the fc weight, summed over the
C=4 context slots, plus bias.

Strategy (BATCH_WAYS x VOCAB_WAYS = 8 cores; 4 x 2 by default):
  - Host: build table t[i, v, o] = fc_w[o, i*V+v] + fc_b[o]/C in BF16, shard
    o into VOCAB_WAYS contiguous column blocks -> per-core contiguous table
    [C*V, V/VOCAB_WAYS] bf16. The correctness gate is rel_err < 2e-2 and the
    bf16 round-off lands ~1e-3, so halving the gathered bytes is free.
  - Device: each core owns B/BATCH_WAYS batch rows and V/VOCAB_WAYS output
    cols. Per 128-row batch block: indirect-DMA gathers (one line-rate
    descriptor per row), a chained DVE bf16 reduction, bf16 store.
    Pair-first issue order: slots 0+1 of every block stream in first so the
    DVE chain starts early.
  - Host: stitch the 8 per-core bf16 outputs into [B, V] f32.
"""

import os

import numpy as np
import ml_dtypes

from concourse import bacc, bass, mybir
import concourse.tile as tile
from concourse.bass_utils import run_bass_kernel_spmd

V = 8192          # vocab (both in and out)
C = 4             # context slots
B = 1024          # batch
M = 8             # cores
P = 128           # SBUF partitions / batch block
R = C * V         # table rows

BF16 = ml_dtypes.bfloat16

BATCH_WAYS = int(os.environ.get("KERNEL_BATCH_WAYS", "4"))
VOCAB_WAYS = M // BATCH_WAYS
BS = B // BATCH_WAYS   # batch rows per core
VS = V // VOCAB_WAYS   # output cols per core
NBLK = BS // P         # 128-row batch blocks per core

_NC_CACHE = None
LAST_RESULTS = None  # test harness reads exec_time_ns from here


def _build_nc():
    nc = bacc.Bacc("TRN2", target_bir_lowering=False, debug=False)
    idx_d = nc.dram_tensor("idx", [BS, C], mybir.dt.int32, kind="ExternalInput")
    tab_d = nc.dram_tensor("tab", [R, VS], mybir.dt.bfloat16, kind="ExternalInput")
    out_d = nc.dram_tensor("out", [BS, VS], mybir.dt.bfloat16, kind="ExternalOutput")

    with tile.TileContext(nc) as tc:
        with tc.tile_pool(name="sbuf", bufs=1) as pool:
            idx_ts, slots, accs = [], [], []
            for blk in range(NBLK):
                row0 = blk * P
                idx_t = pool.tile([P, C], mybir.dt.int32, tag=f"idx{blk}")
                nc.sync.dma_start(out=idx_t[:], in_=idx_d[row0 : row0 + P, :])
                idx_ts.append(idx_t)
                # one tile per (block, slot): no shared-tile WAR deps between
                # late gathers and the DVE reads of earlier slots
                slots.append(
                    [
                        pool.tile(
                            [P, VS],
                            mybir.dt.bfloat16,
                            tag=f"g{blk}_{i}",
                            name=f"g{blk}_{i}",
                        )
                        for i in range(C)
                    ]
                )
                accs.append(
                    pool.tile([P, VS], mybir.dt.bfloat16, tag=f"a{blk}", name=f"a{blk}")
                )

            def gather(blk, i):
                # NB: a multi-column offset AP ([P, C] indices in one op)
                # passes CoreSim but returns garbage on HW — keep [P, 1].
                nc.gpsimd.indirect_dma_start(
                    out=slots[blk][i][:],
                    out_offset=None,
                    in_=tab_d[:],
                    in_offset=bass.IndirectOffsetOnAxis(
                        ap=idx_ts[blk][:, i : i + 1], axis=0
                    ),
                )

            # Pair-first issue: slots 0+1 of each block stream in first so the
            # DVE reduction starts as early as possible.
            for blk in range(NBLK):
                gather(blk, 0)
                gather(blk, 1)
            for blk in range(NBLK):
                nc.vector.tensor_add(
                    out=accs[blk][:], in0=slots[blk][0][:], in1=slots[blk][1][:]
                )
            tail_split = bool(int(os.environ.get("KERNEL_TAIL_SPLIT", "0")))
            last = NBLK - 1
            for i in range(2, C):
                for blk in range(NBLK):
                    gather(blk, i)
                for blk in range(NBLK):
                    if tail_split and i == C - 1 and blk == last:
                        continue  # handled below in halves
                    nc.vector.tensor_add(
                        out=accs[blk][:], in0=accs[blk][:], in1=slots[blk][i][:]
                    )
            for blk in range(NBLK):
                row0 = blk * P
                if tail_split and blk == last:
                    continue
                nc.sync.dma_start(out=out_d[row0 : row0 + P, :], in_=accs[blk][:])
            if tail_split:
                # the last block's final add + store leave the critical path in
                # half-width pieces: store of half 0 overlaps the add of half 1
                row0 = last * P
                vh = VS // 2
                for half in range(2):
                    sl = slice(half * vh, (half + 1) * vh)
                    nc.vector.tensor_add(
                        out=accs[last][:, sl],
                        in0=accs[last][:, sl],
                        in1=slots[last][C - 1][:, sl],
                    )
                    nc.sync.dma_start(
                        out=out_d[row0 : row0 + P, sl], in_=accs[last][:, sl]
                    )
    nc.compile()
    return nc


def _host_prep(contexts, fc_w, fc_b):
    contexts = np.asarray(contexts)
    fc_w = np.asarray(fc_w, dtype=np.float32)
    fc_b = np.asarray(fc_b, dtype=np.float32)
    idx = np.arange(C, dtype=np.int32)[None, :] * V + contexts.astype(np.int32)
    idx = np.ascontiguousarray(idx)

    w3 = fc_w.reshape(V, C, V)  # [o, i, v]
    bias_per_slot = (fc_b / C)[:, None]  # [o, 1]
    vocab_shards = []
    for vw in range(VOCAB_WAYS):
        o_sl = slice(vw * VS, (vw + 1) * VS)
        shard = np.empty((C, V, VS), dtype=BF16)
        tmp = np.empty((V, VS), dtype=np.float32)
        for i in range(C):
            # [o_shard, v].T -> [v, o_shard], fused bias add, then bf16 round
            np.add(w3[o_sl, i, :].T, bias_per_slot[o_sl].T, out=tmp)
            shard[i] = tmp.astype(BF16)
        vocab_shards.append(shard.reshape(R, VS))
    return idx, vocab_shards


def kernel(contexts, fc_w, fc_b):
    global _NC_CACHE, LAST_RESULTS
    idx, vocab_shards = _host_prep(contexts, fc_w, fc_b)
    if _NC_CACHE is None:
        _NC_CACHE = _build_nc()
    nc = _NC_CACHE

    # core m = bw * VOCAB_WAYS + vw owns batch rows [bw*BS:(bw+1)*BS] and
    # output cols [vw*VS:(vw+1)*VS]
    in_maps = []
    for m in range(M):
        bw, vw = divmod(m, VOCAB_WAYS)
        in_maps.append(
            {"idx": idx[bw * BS : (bw + 1) * BS], "tab": vocab_shards[vw]}
        )
    trace = bool(os.environ.get("KERNEL_TRACE"))
    res = run_bass_kernel_spmd(
        nc, in_maps, list(range(M)), trace=trace, stitch_traces=False
    )
    LAST_RESULTS = res

    out = np.empty((B, V), dtype=np.float32)
    for m in range(M):
        bw, vw = divmod(m, VOCAB_WAYS)
        out[bw * BS : (bw + 1) * BS, vw * VS : (vw + 1) * VS] = res.results[m][
            "out"
        ].astype(np.float32)
    return out


# revision 10
# speedup vs baseline: 1.3131x; 1.3131x over previous
"""CBOW embedding-lookup kernel for Trainium2 (8 NeuronCores).

Math: out[b, o] = sum_i fc_w[o, i*V + contexts[b, i]] + fc_b[o]
i.e. a row-gather over a transposed view of the fc weight, summed over the
C=4 context slots, plus bias.

Strategy (pure batch-parallel, 8 cores x 128 batch rows):
  - Host: build table t[i, v, o] = fc_w[o, i*V+v] + fc_b[o]/C in BF16
    ([C*V, V], replicated). The correctness gate is rel_err < 2e-2 and bf16
    round-off lands ~7e-3, so halving gathered bytes is free.
  - Device per core: 4 indirect-DMA gathers of 128 x 16KB rows, chained DVE
    bf16 adds, bf16 store, tail processed in column quarters so the final
    add and the output store pipeline. The kernel is HBM-bound (~400 GB/s
    effective per core); descriptor emission is ~1.1us per 128-row call.
  - Host: stitch per-core bf16 outputs into [B, V] f32.
"""

import os

import numpy as np
import ml_dtypes

from concourse import bacc, bass, mybir
import concourse.tile as tile
from concourse.bass_utils import run_bass_kernel_spmd

V = 8192          # vocab (both in and out)
C = 4             # context slots
B = 1024          # batch
M = 8             # cores
P = 128           # SBUF partitions / batch block
R = C * V         # table rows

BF16 = ml_dtypes.bfloat16

BS = B // M            # batch rows per core (128)
FLAT_IDX = bool(int(os.environ.get("KERNEL_FLAT_IDX", "0")))
TAIL_Q = int(os.environ.get("KERNEL_TAIL_Q", "4"))  # tail column splits
COL_SPLIT = int(os.environ.get("KERNEL_COL_SPLIT", "1"))  # column stripes

_NC_CACHE = None
LAST_RESULTS = None  # test harness reads exec_time_ns from here


def _build_nc():
    nc = bacc.Bacc("TRN2", target_bir_lowering=False, debug=False)
    idx_shape = [C, BS] if FLAT_IDX else [BS, C]
    idx_d = nc.dram_tensor("idx", idx_shape, mybir.dt.int32, kind="ExternalInput")
    tab_d = nc.dram_tensor("tab", [R, V], mybir.dt.bfloat16, kind="ExternalInput")
    out_d = nc.dram_tensor("out", [BS, V], mybir.dt.bfloat16, kind="ExternalOutput")

    with tile.TileContext(nc) as tc:
        with tc.tile_pool(name="sbuf", bufs=1) as pool:
            idx_t = pool.tile(idx_shape, mybir.dt.int32, tag="idx")
            nc.sync.dma_start(out=idx_t[:], in_=idx_d[:])
            slots = [
                pool.tile([P, V], mybir.dt.bfloat16, tag=f"g{i}", name=f"g{i}")
                for i in range(C)
            ]
            acc = pool.tile([P, V], mybir.dt.bfloat16, tag="acc", name="acc")

            def gather(i, sl):
                # NB: non-[P, 1] offset APs (multi-column [P, C], flat
                # [1, P]) pass CoreSim but break on HW — one [P, 1] call
                # per slot. Emission is ~1.1us/call, far from the
                # bottleneck, so the layout costs nothing.
                off = idx_t[i : i + 1, :] if FLAT_IDX else idx_t[:, i : i + 1]
                nc.gpsimd.indirect_dma_start(
                    out=slots[i][:, sl],
                    out_offset=None,
                    in_=tab_d[:],
                    in_offset=bass.IndirectOffsetOnAxis(ap=off, axis=0),
                    # column stripe: gathered row address = idx*V + start col
                    element_offset=sl.start or 0,
                )

            # Column stripes: stripe s's adds/stores overlap stripe s+1's
            # gather drains, so only the last stripe's tail is exposed.
            vw = V // COL_SPLIT
            for s in range(COL_SPLIT):
                col = slice(s * vw, (s + 1) * vw)
                gather(0, col)
                gather(1, col)
                nc.vector.tensor_add(
                    out=acc[:, col], in0=slots[0][:, col], in1=slots[1][:, col]
                )
                gather(2, col)
                gather(3, col)
                nc.vector.tensor_add(
                    out=acc[:, col], in0=acc[:, col], in1=slots[2][:, col]
                )
                # tail: final add + store pipelined in column pieces; only the
                # last stripe's tail is exposed, earlier ones hide behind the
                # next stripe's gather drains — keep them whole (fewer DVE
                # DRAIN overheads)
                tq = TAIL_Q if s == COL_SPLIT - 1 else 1
                vq = vw // tq
                for q in range(tq):
                    sl = slice(s * vw + q * vq, s * vw + (q + 1) * vq)
                    nc.vector.tensor_add(
                        out=acc[:, sl], in0=acc[:, sl], in1=slots[3][:, sl]
                    )
                    nc.sync.dma_start(out=out_d[:, sl], in_=acc[:, sl])
    nc.compile()
    return nc


def _host_prep(contexts, fc_w, fc_b):
    contexts = np.asarray(contexts)
    fc_w = np.asarray(fc_w, dtype=np.float32)
    fc_b = np.asarray(fc_b, dtype=np.float32)
    idx = np.arange(C, dtype=np.int32)[None, :] * V + contexts.astype(np.int32)
    idx = np.ascontiguousarray(idx)  # [B, C]

    w3 = fc_w.reshape(V, C, V)  # [o, i, v]
    bias_per_slot = (fc_b / C)[None, :]  # [1, o]
    tab = np.empty((C, V, V), dtype=BF16)
    tmp = np.empty((V, V), dtype=np.float32)
    for i in range(C):
        # [o, v].T -> [v, o], fused bias add, then bf16 round
        np.add(w3[:, i, :].T, bias_per_slot, out=tmp)
        tab[i] = tmp.astype(BF16)
    return idx, tab.reshape(R, V)


def kernel(contexts, fc_w, fc_b):
    global _NC_CACHE, LAST_RESULTS
    idx, tab = _host_prep(contexts, fc_w, fc_b)
    if _NC_CACHE is None:
        _NC_CACHE = _build_nc()
    nc = _NC_CACHE

    in_maps = []
    for m in range(M):
        core_idx = idx[m * BS : (m + 1) * BS]  # [BS, C]
        if FLAT_IDX:
            core_idx = np.ascontiguousarray(core_idx.T)  # [C, BS]
        in_maps.append({"idx": core_idx, "tab": tab})
    trace = bool(os.environ.get("KERNEL_TRACE"))
    res = run_bass_kernel_spmd(
        nc, in_maps, list(range(M)), trace=trace, stitch_traces=False
    )
    LAST_RESULTS = res

    out = np.empty((B, V), dtype=np.float32)
    for m in range(M):
        out[m * BS : (m + 1) * BS] = res.results[m]["out"].astype(np.float32)
    return out


# revision 12
# speedup vs baseline: 1.3484x; 1.0270x over previous
"""CBOW embedding-lookup kernel for Trainium2 (8 NeuronCores).

Math: out[b, o] = sum_i fc_w[o, i*V + contexts[b, i]] + fc_b[o]
i.e. a row-gather over a transposed view of the fc weight, summed over the
C=4 context slots, plus bias.

Strategy (pure batch-parallel, 8 cores x 128 batch rows):
  - Host: build table t[i, v, o] = fc_w[o, i*V+v] + fc_b[o]/C in BF16
    ([C*V, V], replicated). The correctness gate is rel_err < 2e-2 and bf16
    round-off lands ~7e-3, so halving gathered bytes is free.
  - Device per core: 4 indirect-DMA gathers of 128 x 16KB rows, chained DVE
    bf16 adds, bf16 store, tail processed in column quarters so the final
    add and the output store pipeline. The kernel is HBM-bound (~400 GB/s
    effective per core); descriptor emission is ~1.1us per 128-row call.
  - Host: stitch per-core bf16 outputs into [B, V] f32.
"""

import os

import numpy as np
import ml_dtypes

from concourse import bacc, bass, mybir
import concourse.tile as tile
from concourse.bass_utils import run_bass_kernel_spmd

V = 8192          # vocab (both in and out)
C = 4             # context slots
B = 1024          # batch
M = 8             # cores
P = 128           # SBUF partitions / batch block
R = C * V         # table rows

BF16 = ml_dtypes.bfloat16

BS = B // M            # batch rows per core (128)
FLAT_IDX = bool(int(os.environ.get("KERNEL_FLAT_IDX", "0")))
TAIL_Q = int(os.environ.get("KERNEL_TAIL_Q", "4"))  # tail column splits
COL_SPLIT = int(os.environ.get("KERNEL_COL_SPLIT", "1"))  # column stripes
PSPLIT = int(os.environ.get("KERNEL_PSPLIT", "1"))  # partition-halves per gather

_NC_CACHE = None
LAST_RESULTS = None  # test harness reads exec_time_ns from here


def _build_nc():
    nc = bacc.Bacc("TRN2", target_bir_lowering=False, debug=False)
    idx_shape = [C, BS] if FLAT_IDX else [BS, C]
    idx_d = nc.dram_tensor("idx", idx_shape, mybir.dt.int32, kind="ExternalInput")
    tab_d = nc.dram_tensor("tab", [R, V], mybir.dt.bfloat16, kind="ExternalInput")
    out_d = nc.dram_tensor("out", [BS, V], mybir.dt.bfloat16, kind="ExternalOutput")

    with tile.TileContext(nc) as tc:
        with tc.tile_pool(name="sbuf", bufs=1) as pool:
            idx_t = pool.tile(idx_shape, mybir.dt.int32, tag="idx")
            nc.sync.dma_start(out=idx_t[:], in_=idx_d[:])
            slots = [
                pool.tile([P, V], mybir.dt.bfloat16, tag=f"g{i}", name=f"g{i}")
                for i in range(C)
            ]
            acc = pool.tile([P, V], mybir.dt.bfloat16, tag="acc", name="acc")

            def gather(i, sl):
                # NB: non-[P, 1] offset APs (multi-column [P, C], flat
                # [1, P]) pass CoreSim but break on HW — one [P, 1] call
                # per slot. Emission is ~1.1us/call, far from the
                # bottleneck. Partition-splitting (PSPLIT) keeps 16KB
                # descriptors but doubles the in-flight DMA queues, which
                # measurably raises the SDMA drain rate; the partition
                # swizzle maps row halves to even/odd engines, so a pair
                # of half-calls covers all 16 engines.
                ph = P // PSPLIT
                for h in range(PSPLIT):
                    rows = slice(h * ph, (h + 1) * ph)
                    off = (
                        idx_t[i : i + 1, rows]
                        if FLAT_IDX
                        else idx_t[rows, i : i + 1]
                    )
                    nc.gpsimd.indirect_dma_start(
                        out=slots[i][rows, sl],
                        out_offset=None,
                        in_=tab_d[:],
                        in_offset=bass.IndirectOffsetOnAxis(ap=off, axis=0),
                        # column stripe: row address = idx*V + start col
                        element_offset=sl.start or 0,
                    )

            # Column stripes: stripe s's adds/stores overlap stripe s+1's
            # gather drains, so only the last stripe's tail is exposed.
            vw = V // COL_SPLIT
            for s in range(COL_SPLIT):
                col = slice(s * vw, (s + 1) * vw)
                gather(0, col)
                gather(1, col)
                nc.vector.tensor_add(
                    out=acc[:, col], in0=slots[0][:, col], in1=slots[1][:, col]
                )
                gather(2, col)
                gather(3, col)
                nc.vector.tensor_add(
                    out=acc[:, col], in0=acc[:, col], in1=slots[2][:, col]
                )
                # tail: final add + store pipelined in column pieces; only the
                # last stripe's tail is exposed, earlier ones hide behind the
                # next stripe's gather drains — keep them whole (fewer DVE
                # DRAIN overheads)
                tq = TAIL_Q if s == COL_SPLIT - 1 else 1
                vq = vw // tq
                for q in range(tq):
                    sl = slice(s * vw + q * vq, s * vw + (q + 1) * vq)
                    nc.vector.tensor_add(
                        out=acc[:, sl], in0=acc[:, sl], in1=slots[3][:, sl]
                    )
                    nc.sync.dma_start(out=out_d[:, sl], in_=acc[:, sl])
    nc.compile()
    return nc


def _host_prep(contexts, fc_w, fc_b):
    contexts = np.asarray(contexts)
    fc_w = np.asarray(fc_w, dtype=np.float32)
    fc_b = np.asarray(fc_b, dtype=np.float32)
    idx = np.arange(C, dtype=np.int32)[None, :] * V + contexts.astype(np.int32)
    idx = np.ascontiguousarray(idx)  # [B, C]

    w3 = fc_w.reshape(V, C, V)  # [o, i, v]
    bias_per_slot = (fc_b / C)[None, :]  # [1, o]
    tab = np.empty((C, V, V), dtype=BF16)
    tmp = np.empty((V, V), dtype=np.float32)
    for i in range(C):
        # [o, v].T -> [v, o], fused bias add, then bf16 round
        np.add(w3[:, i, :].T, bias_per_slot, out=tmp)
        tab[i] = tmp.astype(BF16)
    return idx, tab.reshape(R, V)


def kernel(contexts, fc_w, fc_b):
    global _NC_CACHE, LAST_RESULTS
    idx, tab = _host_prep(contexts, fc_w, fc_b)
    if _NC_CACHE is None:
        _NC_CACHE = _build_nc()
    nc = _NC_CACHE

    in_maps = []
    for m in range(M):
        core_idx = idx[m * BS : (m + 1) * BS]  # [BS, C]
        if FLAT_IDX:
            core_idx = np.ascontiguousarray(core_idx.T)  # [C, BS]
        in_maps.append({"idx": core_idx, "tab": tab})
    trace = bool(os.environ.get("KERNEL_TRACE"))
    res = run_bass_kernel_spmd(
        nc, in_maps, list(range(M)), trace=trace, stitch_traces=False
    )
    LAST_RESULTS = res

    out = np.empty((B, V), dtype=np.float32)
    for m in range(M):
        out[m * BS : (m + 1) * BS] = res.results[m]["out"].astype(np.float32)
    return out


# revision 17
# speedup vs baseline: 1.5121x; 1.1213x over previous
"""CBOW embedding-lookup kernel for Trainium2 (8 NeuronCores).

Math: out[b, o] = sum_i fc_w[o, i*V + contexts[b, i]] + fc_b[o]
i.e. a row-gather over a transposed view of the fc weight, summed over the
C=4 context slots, plus bias.

Strategy (pure batch-parallel, 8 cores x 128 batch rows):
  - Host: build table t[i, v, o] = fc_w[o, i*V+v] + fc_b[o]/C in BF16
    ([C*V, V], replicated). The correctness gate is rel_err < 2e-2 and bf16
    round-off lands ~7e-3, so halving gathered bytes is free.
  - Device per core: 4 indirect-DMA gathers of 128 x 16KB rows, chained DVE
    bf16 adds, bf16 store, tail processed in column quarters so the final
    add and the output store pipeline. The kernel is HBM-bound (~400 GB/s
    effective per core); descriptor emission is ~1.1us per 128-row call.
  - Host: stitch per-core bf16 outputs into [B, V] f32.
"""

import os

import numpy as np
import ml_dtypes

from concourse import bacc, bass, mybir
import concourse.tile as tile
from concourse.bass_utils import run_bass_kernel_spmd

V = 8192          # vocab (both in and out)
C = 4             # context slots
B = 1024          # batch
M = 8             # cores
P = 128           # SBUF partitions / batch block
R = C * V         # table rows

BF16 = ml_dtypes.bfloat16

BS = B // M            # batch rows per core (128)
FLAT_IDX = bool(int(os.environ.get("KERNEL_FLAT_IDX", "0")))
TAIL_Q = int(os.environ.get("KERNEL_TAIL_Q", "4"))  # tail column splits
COL_SPLIT = int(os.environ.get("KERNEL_COL_SPLIT", "1"))  # column stripes
PSPLIT = int(os.environ.get("KERNEL_PSPLIT", "1"))  # partition-halves per gather
IDX_SCALAR = bool(int(os.environ.get("KERNEL_IDX_SCALAR", "0")))  # idx via ACT ring
TAIL_HALF = bool(int(os.environ.get("KERNEL_TAIL_HALF", "0")))  # slot3 in halves

_NC_CACHE = None
LAST_RESULTS = None  # test harness reads exec_time_ns from here


def _build_nc():
    nc = bacc.Bacc("TRN2", target_bir_lowering=False, debug=False)
    idx_shape = [C, BS] if FLAT_IDX else [BS, C]
    idx_d = nc.dram_tensor("idx", idx_shape, mybir.dt.int32, kind="ExternalInput")
    tab_d = nc.dram_tensor("tab", [R, V], mybir.dt.bfloat16, kind="ExternalInput")
    out_d = nc.dram_tensor("out", [BS, V], mybir.dt.bfloat16, kind="ExternalOutput")

    with tile.TileContext(nc) as tc:
        with tc.tile_pool(name="sbuf", bufs=1) as pool:
            idx_t = pool.tile(idx_shape, mybir.dt.int32, tag="idx")
            # the Scalar (ACT) HWDGE ring is idle at kernel start; Sync's is
            # behind a post-preamble drain, costing ~1.4us before the first
            # gather can see the indices
            idx_eng = nc.scalar if IDX_SCALAR else nc.sync
            idx_eng.dma_start(out=idx_t[:], in_=idx_d[:])
            slots = [
                pool.tile([P, V], mybir.dt.bfloat16, tag=f"g{i}", name=f"g{i}")
                for i in range(C)
            ]
            acc = pool.tile([P, V], mybir.dt.bfloat16, tag="acc", name="acc")

            def gather(i, sl):
                # NB: non-[P, 1] offset APs (multi-column [P, C], flat
                # [1, P]) pass CoreSim but break on HW — one [P, 1] call
                # per slot. Emission is ~1.1us/call, far from the
                # bottleneck. Partition-splitting (PSPLIT) keeps 16KB
                # descriptors but doubles the in-flight DMA queues, which
                # measurably raises the SDMA drain rate; the partition
                # swizzle maps row halves to even/odd engines, so a pair
                # of half-calls covers all 16 engines.
                ph = P // PSPLIT
                for h in range(PSPLIT):
                    rows = slice(h * ph, (h + 1) * ph)
                    off = (
                        idx_t[i : i + 1, rows]
                        if FLAT_IDX
                        else idx_t[rows, i : i + 1]
                    )
                    nc.gpsimd.indirect_dma_start(
                        out=slots[i][rows, sl],
                        out_offset=None,
                        in_=tab_d[:],
                        in_offset=bass.IndirectOffsetOnAxis(ap=off, axis=0),
                        # column stripe: row address = idx*V + start col
                        element_offset=sl.start or 0,
                    )

            # Column stripes: stripe s's adds/stores overlap stripe s+1's
            # gather drains, so only the last stripe's tail is exposed.
            vw = V // COL_SPLIT
            for s in range(COL_SPLIT):
                col = slice(s * vw, (s + 1) * vw)
                gather(0, col)
                gather(1, col)
                nc.vector.tensor_add(
                    out=acc[:, col], in0=slots[0][:, col], in1=slots[1][:, col]
                )
                gather(2, col)
                if not TAIL_HALF:
                    gather(3, col)
                nc.vector.tensor_add(
                    out=acc[:, col], in0=acc[:, col], in1=slots[2][:, col]
                )
                if TAIL_HALF:
                    continue
                # tail: final add + store pipelined in column pieces; only the
                # last stripe's tail is exposed, earlier ones hide behind the
                # next stripe's gather drains — keep them whole (fewer DVE
                # DRAIN overheads)
                tq = TAIL_Q if s == COL_SPLIT - 1 else 1
                vq = vw // tq
                for q in range(tq):
                    sl = slice(s * vw + q * vq, s * vw + (q + 1) * vq)
                    nc.vector.tensor_add(
                        out=acc[:, sl], in0=acc[:, sl], in1=slots[3][:, sl]
                    )
                    nc.sync.dma_start(out=out_d[:, sl], in_=acc[:, sl])

            if TAIL_HALF:
                # slot 3 gathered as two column-half calls (own tiles, clean
                # deps): the left half's final adds + stores run while the
                # right half is still draining, so only the right half's tail
                # is exposed after the last gather byte
                assert COL_SPLIT == 1
                vh = V // 2
                g3h = [
                    pool.tile(
                        [P, vh], mybir.dt.bfloat16, tag=f"g3h{h}", name=f"g3h{h}"
                    )
                    for h in range(2)
                ]
                off3 = idx_t[3:4, :] if FLAT_IDX else idx_t[:, 3:4]
                for h in range(2):
                    nc.gpsimd.indirect_dma_start(
                        out=g3h[h][:],
                        out_offset=None,
                        in_=tab_d[:],
                        in_offset=bass.IndirectOffsetOnAxis(ap=off3, axis=0),
                        element_offset=h * vh,
                    )
                for h in range(2):
                    npieces = 2 if h == 0 else TAIL_Q
                    pw = vh // npieces
                    for q in range(npieces):
                        lo = h * vh + q * pw
                        sl = slice(lo, lo + pw)
                        nc.vector.tensor_add(
                            out=acc[:, sl],
                            in0=acc[:, sl],
                            in1=g3h[h][:, q * pw : (q + 1) * pw],
                        )
                        nc.sync.dma_start(out=out_d[:, sl], in_=acc[:, sl])
    nc.compile()
    return nc


def _host_prep(contexts, fc_w, fc_b):
    contexts = np.asarray(contexts)
    fc_w = np.asarray(fc_w, dtype=np.float32)
    fc_b = np.asarray(fc_b, dtype=np.float32)
    idx = np.arange(C, dtype=np.int32)[None, :] * V + contexts.astype(np.int32)
    idx = np.ascontiguousarray(idx)  # [B, C]

    w3 = fc_w.reshape(V, C, V)  # [o, i, v]
    bias_per_slot = (fc_b / C)[None, :]  # [1, o]
    tab = np.empty((C, V, V), dtype=BF16)
    tmp = np.empty((V, V), dtype=np.float32)
    for i in range(C):
        # [o, v].T -> [v, o], fused bias add, then bf16 round
        np.add(w3[:, i, :].T, bias_per_slot, out=tmp)
        tab[i] = tmp.astype(BF16)
    return idx, tab.reshape(R, V)


def kernel(contexts, fc_w, fc_b):
    global _NC_CACHE, LAST_RESULTS
    idx, tab = _host_prep(contexts, fc_w, fc_b)
    if _NC_CACHE is None:
        _NC_CACHE = _build_nc()
    nc = _NC_CACHE

    in_maps = []
    for m in range(M):
        core_idx = idx[m * BS : (m + 1) * BS]  # [BS, C]
        if FLAT_IDX:
            core_idx = np.ascontiguousarray(core_idx.T)  # [C, BS]
        in_maps.append({"idx": core_idx, "tab": tab})
    trace = bool(os.environ.get("KERNEL_TRACE"))
    res = run_bass_kernel_spmd(
        nc, in_maps, list(range(M)), trace=trace, stitch_traces=False
    )
    LAST_RESULTS = res

    out = np.empty((B, V), dtype=np.float32)
    for m in range(M):
        out[m * BS : (m + 1) * BS] = res.results[m]["out"].astype(np.float32)
    return out
